# revision 22
# baseline (speedup 1.0000x reference)
"""Trainium2 Bass kernel for DBFLinear:
    y = ((x * s0) @ unpack(bp1).T * s2) @ unpack(bp3).T * s4 + bias

Strategy (v2, fused weights): since W1/W3 are +/-1, precompute on device
    W13[i, o] = sum_m W1[m, i] * s2[m] * W3[o, m]        (build GEMM)
    y = x @ (s0[:, None] * W13) * s4 + bias              (main GEMM)
Total FLOPs drop from 2*B*IN*MID + 2*B*MID*OUT to
IN*MID*OUT (build, sharded) + 2*B*IN*OUT (main) -- 25% less PE work.

Sharding: each core owns 512 output columns (oc): it builds its W13
slice (contraction over m, stat = s2-scaled W3^T tiles, moving = W1
unpacked in natural layout -- no W1 transpose needed) and then runs the
main GEMM over the full batch (moving = host-transposed x^T windows).
The host assembles y from the 8 column shards (one transpose each).

LD_WEIGHTS amortization: hardware pays ~128 PE rows per stationary
load, unhidden.  Both GEMMs therefore reuse each loaded stationary
across multiple 512-row moving passes (2 in build, 4 in main; main
splits the i-contraction in halves with an SBUF fp16 partial-sum add
so 4 PSUM banks suffice per accumulation group).

SBUF: build-phase tiles live in a scoped pool released before the main
phase; x^T window buffers alternate between an early pool (prefetch
during build) and a pool carved from the released build zone.
"""

import sys

import numpy as np

sys.path.insert(0, "/opt/trn_rl_repo")

import concourse.bass as bass
import concourse.mybir as mybir
import concourse.tile as tile
from concourse import bacc
from concourse.bass_utils import run_bass_kernel_spmd

N_CORES = 8
B, IN, MID, OUT = 8192, 4096, 4096, 4096
P = 128
OC = OUT // N_CORES      # 512 output cols per core
N_OB = OC // P           # 4 stationary col-blocks
IK = IN // P             # 32 i-blocks
MK = MID // P            # 32 m-blocks
NCHUNK = 4               # build i-chunks of 1024
CH = IN // NCHUNK        # 1024
NWP = 4                  # main batch windows of 2048
WB = B // NWP            # 2048
NSP = WB // 512          # 4 spans of 512 per window
N_WARM = 208  # x2 PE instructions = 26 full hw-decode bundles


def build_program():
    nc = bacc.Bacc(num_devices=N_CORES)
    f16, f32, i32 = mybir.dt.float16, mybir.dt.float32, mybir.dt.int32
    Act = mybir.ActivationFunctionType

    xT_d = nc.dram_tensor("xT", [IN, B], f16, kind="ExternalInput")
    bp1_d = nc.dram_tensor("bp1", [MID, IN // 8], i32, kind="ExternalInput")
    w3st_d = nc.dram_tensor("w3st", [MID, OC], f16, kind="ExternalInput")
    mask_d = nc.dram_tensor("mask", [P, 8], i32, kind="ExternalInput")
    s0r_d = nc.dram_tensor("s0rep", [P, IN], f16, kind="ExternalInput")
    s4_d = nc.dram_tensor("s4", [P, N_OB], f32, kind="ExternalInput")
    bias_d = nc.dram_tensor("bias", [P, N_OB], f32, kind="ExternalInput")
    yT_d = nc.dram_tensor("yT", [OC, B], f16, kind="ExternalOutput")

    xTv = xT_d.rearrange("(h k p) b -> h p k b", p=P, k=IK // 2)  # [2,128,16,B]
    yv = yT_d.rearrange("(g p) b -> p g b", p=P)                  # [128,4,B]
    w3v = w3st_d.rearrange("(q kb p) o -> q p kb o", p=P, kb=8)   # [4,128,8,OC]
    bp1p = bp1_d.rearrange("(kb two p) b -> kb p two b", p=P, two=2)  # [16,128,2,512]

    with tile.TileContext(nc) as tc:
        with (
            tc.tile_pool(name="consts", bufs=1) as consts,
            tc.tile_pool(name="wkP", bufs=1) as wkP,
            tc.tile_pool(name="xwpE", bufs=1) as xwpE,
            tc.tile_pool(name="psum", bufs=8, space="PSUM") as psum,
        ):
            mask_t = consts.tile([P, 8], i32)
            s4_t = consts.tile([P, N_OB], f32)
            bias_t = consts.tile([P, N_OB], f32)
            neg_half = consts.tile([P, 1], f32)
            nc.sync.dma_start(mask_t[:], mask_d[:])
            for t, d in ((s4_t, s4_d), (bias_t, bias_d)):
                nc.gpsimd.dma_start(t[:], d[:])
            nc.vector.memset(neg_half[:], -0.5)

            w13 = wkP.tile([P, IK, OC], f16, name="w13")    # 32KB/part

            _ps_n = [0]

            def ps_tile():
                _ps_n[0] += 1
                return psum.tile([P, 512], f32, tag="ps",
                                 name=f"ps{_ps_n[0]}")

            # Warm the PE HAM clock gate while W3 prep fills the pipeline.
            junk_t = consts.tile([P, 16], f16)
            nc.vector.memset(junk_t[:], 0.5)
            junk = junk_t[:]
            warm_ps = ps_tile()
            for _ in range(N_WARM):
                nc.tensor.matmul(warm_ps[:16, :16], junk, junk,
                                 start=True, stop=True)

            xw = {}

            def xw_load(wp, half, pool, tag, eng=None):
                t = pool.tile([P, IK // 2, WB], f16, tag=tag, bufs=1,
                              name=f"xw_{wp}_{half}")
                (eng or nc.scalar).dma_start(
                    t[:], xTv[half, :, :, wp * WB:(wp + 1) * WB])
                xw[(wp, half)] = t

            with tc.tile_pool(name="wkB", bufs=1) as wk:
                s0r_t = wk.tile([P, IN], f16, name="s0r_t")
                w3sTq = [wk.tile([P, 8, OC], f16, name=f"w3sTq{q}")
                         for q in range(4)]  # 4 x 8KB/part
                # Host-prepared s2-scaled W3^T shard: straight loads.
                for q in range(4):
                    nc.sync.dma_start(w3sTq[q][:], w3v[q])

                # bp1 block loads ride the ACT ring, triggered 4 ahead.
                byt1s = {}

                def byt1_load(c, kb, eng):
                    t = wk.tile([P, 2, CH // 8], i32, tag="byt1", bufs=5,
                                name=f"byt1_{c}_{kb}")
                    eng.dma_start(
                        t[:], bp1p[kb, :, :, c * 128:(c + 1) * 128])
                    byt1s[(c, kb)] = t

                for kb0 in range(2):
                    byt1_load(0, kb0, nc.scalar)

                # First x^T window rides the idle gpsimd engine; s0rep
                # follows it there (needed only at the first epilogue).
                xw_load(0, 0, xwpE, "xwE", eng=nc.gpsimd)
                nc.gpsimd.dma_start(s0r_t[:], s0r_d[:])

                # -- Build GEMM: W13^T chunks, scale by s0, transpose. --
                # Chunk epilogues are emitted after the next chunk's first
                # unpacks so the ACT queue never blocks the unpack pipeline.
                w1us = {}

                def unpack(c, m):
                    if m % 2 == 0:
                        kbpf = m // 2 + 2
                        pf = (c, kbpf) if kbpf < MK // 2 \
                            else (c + 1, kbpf - MK // 2)
                        if pf[0] < NCHUNK and pf not in byt1s:
                            byt1_load(pf[0], pf[1], nc.scalar)
                    byt1 = byt1s[(c, m // 2)]
                    if m % 2 == 1:
                        byt1s.pop((c, m // 2))
                    masked = wk.tile([P, CH], i32, tag="masked1", bufs=6,
                                     name=f"masked1_{c}_{m}")
                    in0 = byt1[:, m % 2, :][:, :, None] \
                        .broadcast_to([P, 128, 8])
                    in1 = mask_t[:][:, None, :].broadcast_to([P, 128, 8])
                    nc.vector.tensor_tensor(
                        masked[:].rearrange("p (b j) -> p b j", j=8),
                        in0, in1, mybir.AluOpType.bitwise_and)
                    w1u = wk.tile([P, CH], f16, tag="w1u", bufs=8,
                                  name=f"w1u_{c}_{m}")
                    nc.scalar.activation(w1u[:], masked[:], Act.Sign,
                                         bias=neg_half[:, 0:1])
                    w1us[(c, m)] = w1u

                def make_epilogue(c, psB):
                    def epi():
                        w13T = wk.tile([P, N_OB, CH], f16, tag="w13T",
                                       bufs=1, name=f"w13T_{c}")
                        for ob in range(N_OB):
                            for w in range(2):
                                nc.scalar.activation(
                                    w13T[:, ob, w * 512:(w + 1) * 512],
                                    psB[ob * 2 + w][:], Act.Copy)
                        nc.gpsimd.tensor_tensor(
                            w13T[:],
                            w13T[:],
                            s0r_t[:, c * CH:(c + 1) * CH][:, None, :]
                            .broadcast_to([P, N_OB, CH]),
                            mybir.AluOpType.mult)
                        for ob in range(N_OB):
                            nc.sync.dma_start_transpose(
                                w13[:, c * 8:(c + 1) * 8,
                                    ob * P:(ob + 1) * P],
                                w13T[:, ob, :])
                    return epi

                pend_epi = None
                for c in range(NCHUNK):
                    for m in range(4):
                        unpack(c, m)
                    if pend_epi is not None:
                        pend_epi()
                        pend_epi = None
                    psB = [ps_tile() for _ in range(8)]
                    for m in range(MK):
                        if m + 4 < MK:
                            unpack(c, m + 4)
                        w1u = w1us.pop((c, m))
                        for ob in range(N_OB):
                            stat = w3sTq[m // 8][:, m % 8,
                                                 ob * P:(ob + 1) * P]
                            for w in range(2):
                                nc.tensor.matmul(
                                    psB[ob * 2 + w][:],
                                    stat,
                                    w1u[:, w * 512:(w + 1) * 512],
                                    start=(m == 0), stop=(m == MK - 1))
                    pend_epi = make_epilogue(c, psB)
                pend_epi()

            # ---- Main GEMM over x^T windows. ----
            with tc.tile_pool(name="wkM", bufs=1) as wkM:
                xw_load(0, 1, wkM, "xwL")
                for wp in range(NWP):
                    ypart = wkM.tile([P, N_OB, WB], f16, tag="ypart", bufs=1,
                                     name=f"ypart_{wp}")
                    for half in range(2):
                        k_idx = wp * 2 + half
                        nxt = (wp, 1) if half == 0 else (wp + 1, 0)
                        if nxt[0] < NWP and nxt not in xw:
                            npool, ntag = ((xwpE, "xwE") if (nxt[0] * 2 + nxt[1]) % 2 == 0
                                           else (wkM, "xwL"))
                            xw_load(nxt[0], nxt[1], npool, ntag)
                        xt = xw[(wp, half)]
                        for ob in range(N_OB):
                            psM = [ps_tile() for _ in range(NSP)]
                            for k in range(IK // 2):
                                stat = w13[:, half * 16 + k,
                                           ob * P:(ob + 1) * P]
                                for sp in range(NSP):
                                    nc.tensor.matmul(
                                        psM[sp][:],
                                        stat,
                                        xt[:, k, sp * 512:(sp + 1) * 512],
                                        start=(k == 0),
                                        stop=(k == IK // 2 - 1))
                            if half == 0:
                                for sp in range(NSP):
                                    nc.scalar.activation(
                                        ypart[:, ob, sp * 512:(sp + 1) * 512],
                                        psM[sp][:], Act.Identity,
                                        bias=bias_t[:, ob:ob + 1],
                                        scale=s4_t[:, ob:ob + 1])
                            else:
                                ystage = wkM.tile([P, WB], f16, tag="ystage",
                                                  bufs=2,
                                                  name=f"ystage_{wp}_{ob}")
                                for sp in range(NSP):
                                    ytmp = wkM.tile([P, 512], f16, tag="ytmp",
                                                    bufs=2,
                                                    name=f"ytmp_{wp}_{ob}_{sp}")
                                    nc.scalar.activation(
                                        ytmp[:], psM[sp][:], Act.Copy,
                                        scale=s4_t[:, ob:ob + 1])
                                    nc.vector.tensor_tensor(
                                        ystage[:, sp * 512:(sp + 1) * 512],
                                        ytmp[:],
                                        ypart[:, ob, sp * 512:(sp + 1) * 512],
                                        mybir.AluOpType.add)
                                nc.sync.dma_start(
                                    yv[:, ob, wp * WB:(wp + 1) * WB],
                                    ystage[:])

    nc.compile()
    return nc


def make_in_maps(x, scaling0, bp1, scaling2, bp3, scaling4, bias,
                 n_cores=N_CORES):
    mask = (1 << (7 - np.arange(8, dtype=np.int32)))[None, :].repeat(P, 0)
    mask = np.ascontiguousarray(mask.astype(np.int32))

    def pcol(v):
        return np.ascontiguousarray(v.astype(np.float32).reshape(-1, P).T)

    xT = np.ascontiguousarray(np.asarray(x, np.float16).T)
    bp1 = np.ascontiguousarray(bp1.reshape(MID, IN // 8))
    bp3 = np.ascontiguousarray(bp3.reshape(OUT, MID // 8))
    s0rep = np.ascontiguousarray(
        np.broadcast_to(scaling0.astype(np.float16)[None, :], (P, IN)))
    shared = {"xT": xT, "bp1": bp1, "mask": mask, "s0rep": s0rep}
    bits3 = np.unpackbits(bp3.astype(np.uint8), axis=1)  # [OUT, MID] 0/1
    w3sT = ((bits3.T.astype(np.int8) * 2 - 1).astype(np.float16)
            * scaling2.astype(np.float16)[:, None])          # [MID, OUT]
    maps = []
    for c in range(n_cores):
        sl = slice(c * OC, (c + 1) * OC)
        maps.append({
            "w3st": np.ascontiguousarray(w3sT[:, sl]),
            "s4": pcol(scaling4[sl]),
            "bias": pcol(bias[sl]),
            **shared,
        })
    return maps


_PROGRAM_CACHE = {}


def run(x, scaling0, bp1, scaling2, bp3, scaling4, bias, **spmd_kwargs):
    """Compile (cached) + run on 8 cores; returns (y, BassKernelResults)."""
    if "nc" not in _PROGRAM_CACHE:
        _PROGRAM_CACHE["nc"] = build_program()
    nc = _PROGRAM_CACHE["nc"]
    in_maps = make_in_maps(x, scaling0, bp1, scaling2, bp3, scaling4, bias)
    res = run_bass_kernel_spmd(nc, in_maps, core_ids=list(range(N_CORES)),
                               **spmd_kwargs)
    y = np.empty((x.shape[0], scaling4.shape[0]), dtype=np.float16)
    for c in range(N_CORES):
        y[:, c * OC:(c + 1) * OC] = res.results[c]["yT"].T
    return y, res


def kernel(x, scaling0, bp1, scaling2, bp3, scaling4, bias):
    y, _ = run(x, scaling0, bp1, scaling2, bp3, scaling4, bias)
    return y
